# revision 2
# baseline (speedup 1.0000x reference)
"""BinaryAttention Trainium2 kernel: data-parallel over batch on 8 NeuronCores.

Per-core pipeline (16 batch items):
  qkvT = W^T-split-bf16 x3 matmul (q,k transposed d-major; v token-major)
  sign/abs via ScalarE from PSUM; per-(b,h) scale c = mean|q| mean|k| /8
  S = sign(q)@sign(k)^T exact in bf16; logits = c*S + bias (DVE)
  e = exp(logits) with fused row-sum; pq = round(255*e/Z) (RNE via +-2^23)
  pqT via PE transpose, scaled by 1/s_v during PSUM evac (ScalarE)
  attnT = v_int @ pqT ; proj folds 1/255 into weights.
"""
import numpy as np
import ml_dtypes

import concourse.bacc as bacc
import concourse.mybir as mybir
from concourse.tile import TileContext
from concourse.bass_utils import run_bass_kernel_spmd
from concourse.bass import AP
import concourse.bass as bass

N_CORES = 8
B = 128
BP = B // N_CORES          # 16 batch items per core
NT = 197                   # tokens
DIM = 768
NH = 12
HD = 64
NREL = 732
TOK = BP * NT              # 3152
F32 = mybir.dt.float32
BF16 = mybir.dt.bfloat16
bf = ml_dtypes.bfloat16
EXP2_23 = 8388608.0
C0 = 1.0 / (NT * HD) / (NT * HD) / 8.0

_CACHE = {}


def _build_nc():
    nc = bacc.Bacc("TRN2", target_bir_lowering=False, debug=False, num_devices=1)
    d = {}
    d["xh"] = nc.dram_tensor("xh", [DIM, TOK], BF16, kind="ExternalInput").ap()
    d["xl"] = nc.dram_tensor("xl", [DIM, TOK], BF16, kind="ExternalInput").ap()
    d["wh"] = nc.dram_tensor("wh", [DIM, 3 * DIM], BF16, kind="ExternalInput").ap()
    d["wl"] = nc.dram_tensor("wl", [DIM, 3 * DIM], BF16, kind="ExternalInput").ap()
    d["pw"] = nc.dram_tensor("pw", [DIM, DIM], BF16, kind="ExternalInput").ap()
    d["pb"] = nc.dram_tensor("pb", [DIM], F32, kind="ExternalInput").ap()
    d["bias"] = nc.dram_tensor("bias", [NH, NT, NT], F32, kind="ExternalInput").ap()
    d["sel"] = nc.dram_tensor("sel", [128, 2], F32, kind="ExternalInput").ap()
    d["ident"] = nc.dram_tensor("ident", [128, 128], BF16, kind="ExternalInput").ap()
    d["out"] = nc.dram_tensor("out", [TOK, DIM], F32, kind="ExternalOutput").ap()
    cscr = nc.dram_tensor("cscr", [BP, 12], F32)

    with TileContext(nc) as tc:
        with (
            tc.tile_pool(name="singles", bufs=1) as singles,
            tc.tile_pool(name="xpool", bufs=2) as xpool,
            tc.tile_pool(name="bpool", bufs=2) as bpool,
            tc.tile_pool(name="hpool", bufs=4) as hpool,
            tc.tile_pool(name="psA", bufs=2, space="PSUM") as psA,
            tc.tile_pool(name="psS", bufs=2, space="PSUM") as psS,
            tc.tile_pool(name="psT", bufs=2, space="PSUM") as psT,
            tc.tile_pool(name="psP", bufs=2, space="PSUM") as psP,
        ):
            # ---- resident weights/constants ----
            whs = singles.tile([128, 6, 3 * DIM], BF16, tag="whs")
            wls = singles.tile([128, 6, 3 * DIM], BF16, tag="wls")
            nc.sync.dma_start(out=whs[:], in_=d["wh"].rearrange("(k p) n -> p k n", p=128))
            nc.sync.dma_start(out=wls[:], in_=d["wl"].rearrange("(k p) n -> p k n", p=128))
            pws = singles.tile([128, 6, DIM], BF16, tag="pws")
            nc.sync.dma_start(out=pws[:], in_=d["pw"].rearrange("(k p) n -> p k n", p=128))
            bias0 = singles.tile([128, NH, NT], F32, tag="bias0")
            bias1 = singles.tile([128, NH, NT], F32, tag="bias1")
            nc.sync.dma_start(out=bias0[:], in_=d["bias"][:, 0:128, :].rearrange("h n m -> n h m"))
            nc.sync.dma_start(out=bias1[:69], in_=d["bias"][:, 128:NT, :].rearrange("h n m -> n h m"))
            pbs = singles.tile([128, DIM], F32, tag="pbs")
            nc.gpsimd.dma_start(out=pbs[:], in_=AP(tensor=d["pb"].tensor, offset=0, ap=[[0, 128], [1, DIM]]))
            sels = singles.tile([128, 2], F32, tag="sels")
            nc.sync.dma_start(out=sels[:], in_=d["sel"])
            idents = singles.tile([128, 128], BF16, tag="idents")
            nc.sync.dma_start(out=idents[:], in_=d["ident"])

            biasn = [bias0, bias1]
            ntl = [128, 69]   # n-tile sizes
            noff = [0, 128]

            for bb in range(BP // 2):   # pairs of batch items
                c2 = 2 * NT
                xh_t = xpool.tile([128, 6, c2], BF16, tag="xh")
                xl_t = xpool.tile([128, 6, c2], BF16, tag="xl")
                nc.sync.dma_start(out=xh_t[:], in_=d["xh"].rearrange("(k p) t -> p k t", p=128)[:, :, bb * c2:(bb + 1) * c2])
                nc.sync.dma_start(out=xl_t[:], in_=d["xl"].rearrange("(k p) t -> p k t", p=128)[:, :, bb * c2:(bb + 1) * c2])

                sgn = [bpool.tile([128, NH, NT], BF16, tag=f"sgn{i}", name=f"sgn{i}") for i in range(2)]
                absc = [bpool.tile([128, 12], F32, tag=f"absc{i}", name=f"absc{i}") for i in range(2)]
                dump = [bpool.tile([128, NT], F32, tag=f"dump{i}", name=f"dump{i}") for i in range(2)]

                # ---- stage A: q,k transposed (12 j-tiles of 128 rows) ----
                for j in range(12):
                    pa = psA.tile([128, c2], F32, tag="A")
                    for k in range(6):
                        wj = slice(j * 128, (j + 1) * 128)
                        first = (k == 0)
                        nc.tensor.matmul(pa[:], whs[:, k, wj], xh_t[:, k, :], start=first, stop=False)
                        nc.tensor.matmul(pa[:], whs[:, k, wj], xl_t[:, k, :], start=False, stop=False)
                        nc.tensor.matmul(pa[:], wls[:, k, wj], xh_t[:, k, :], start=False, stop=(k == 5))
                    for i in range(2):
                        sl = slice(i * NT, (i + 1) * NT)
                        nc.scalar.activation(out=sgn[i][:, j, :], in_=pa[:, sl], func=mybir.ActivationFunctionType.Sign)
                        nc.scalar.activation(out=dump[i][:], in_=pa[:, sl], func=mybir.ActivationFunctionType.Abs,
                                             accum_out=absc[i][:, j:j + 1])

                for i in range(2):
                    b = bb * 2 + i
                    tb = b * NT   # token offset of this batch item in TOK
                    # ---- v natural + quantization ----
                    vI = [bpool.tile([128, DIM], BF16, tag=f"vI{t}", name=f"vI{t}") for t in range(2)]
                    rs = [bpool.tile([128, 12], F32, tag=f"rs{t}", name=f"rs{t}") for t in range(2)]
                    for t in range(2):
                        tn = ntl[t]
                        xoff = i * NT + noff[t]
                        pv = psA.tile([128, 384], F32, tag="A")
                        vq32 = bpool.tile([128, 384], F32, tag="vq32")
                        vmax = bpool.tile([128, 12], F32, tag="vmax")
                        ss = bpool.tile([128, 12], F32, tag="ss")
                        for ch in range(2):
                            vj = slice(1536 + ch * 384, 1536 + (ch + 1) * 384)
                            for k in range(6):
                                first = (k == 0)
                                nc.tensor.matmul(pv[:tn], xh_t[:, k, xoff:xoff + tn], whs[:, k, vj], start=first, stop=False)
                                nc.tensor.matmul(pv[:tn], xl_t[:, k, xoff:xoff + tn], whs[:, k, vj], start=False, stop=(k == 5))
                            hs = slice(ch * 6, (ch + 1) * 6)
                            # clip to [-2,2]
                            nc.vector.tensor_scalar(out=vq32[:tn], in0=pv[:tn], scalar1=2.0, scalar2=-2.0,
                                                    op0=mybir.AluOpType.min, op1=mybir.AluOpType.max)
                            # row max |.| per head
                            nc.vector.tensor_reduce(out=vmax[:tn, hs], in_=vq32[:tn].rearrange("p (h d) -> p h d", h=6),
                                                    axis=mybir.AxisListType.X, op=mybir.AluOpType.max,
                                                    apply_absolute_value=True)
                            # rs = (max+1e-8)/127 ; s = 127/(max+1e-8)
                            nc.vector.tensor_scalar(out=rs[t][:tn, hs], in0=vmax[:tn, hs], scalar1=1e-8, scalar2=1.0 / 127.0,
                                                    op0=mybir.AluOpType.add, op1=mybir.AluOpType.mult)
                            nc.vector.reciprocal(out=ss[:tn, hs], in_=rs[t][:tn, hs])
                            # v*s, round via +-2^23, cast bf16 (exact ints)
                            sbase = ss[:tn, hs]
                            sbc = AP(tensor=sbase.tensor, offset=sbase.offset,
                                     ap=[[int(s), int(c)] for s, c in sbase.ap] + [[0, HD]])
                            v3 = vq32[:tn].rearrange("p (h d) -> p h d", h=6)
                            nc.vector.tensor_tensor(out=v3, in0=v3, in1=sbc,
                                                    op=mybir.AluOpType.mult)
                            nc.vector.tensor_scalar(out=vI[t][:tn, ch * 384:(ch + 1) * 384],
                                                    in0=vq32[:tn], scalar1=EXP2_23, scalar2=EXP2_23,
                                                    op0=mybir.AluOpType.add, op1=mybir.AluOpType.subtract)
                    # ---- c stats ----
                    cst = psS.tile([2, 12], F32, tag="S")
                    nc.tensor.matmul(cst[:], sels[:], absc[i][:], start=True, stop=True)
                    css = bpool.tile([2, 12], F32, tag="css")
                    nc.vector.tensor_copy(css[:], cst[:])
                    csb = bpool.tile([2, 6], F32, tag="csb")
                    nc.vector.tensor_tensor(out=csb[:], in0=css[:2, 0:6], in1=css[:2, 6:12], op=mybir.AluOpType.mult)
                    nc.vector.tensor_scalar_mul(csb[:], csb[:], C0)
                    nc.sync.dma_start(out=cscr.ap()[b].rearrange("(r j) -> r j", r=2), in_=csb[:])
                    cbc = bpool.tile([128, 12], F32, tag="cbc")
                    nc.gpsimd.dma_start(out=cbc[:], in_=AP(tensor=cscr, offset=b * 12, ap=[[0, 128], [1, 12]]))

                    attnT = bpool.tile([128, 6, NT], BF16, tag="attnT")
                    # ---- attention per head ----
                    for h in range(12):
                        jq, base = h // 2, (h % 2) * 64
                        cidx = (h % 2) * 6 + h // 2
                        pqb = [hpool.tile([128, NT], BF16, tag=f"pqb{t}", name=f"pqb{t}") for t in range(2)]
                        pqTs = [hpool.tile([128, NT], BF16, tag=f"pqTs{t}", name=f"pqTs{t}") for t in range(2)]
                        for t in range(2):
                            tn = ntl[t]
                            ps = psS.tile([128, NT], F32, tag="S")
                            nc.tensor.matmul(ps[:tn], sgn[i][base:base + 64, jq, noff[t]:noff[t] + tn],
                                             sgn[i][base:base + 64, 6 + jq, :], start=True, stop=True)
                            lg = hpool.tile([128, NT], F32, tag=f"lg{t}")
                            nc.vector.scalar_tensor_tensor(out=lg[:tn], in0=ps[:tn], scalar=cbc[:tn, cidx:cidx + 1],
                                                           in1=biasn[t][:tn, h, :], op0=mybir.AluOpType.mult,
                                                           op1=mybir.AluOpType.add)
                            ee = hpool.tile([128, NT], F32, tag=f"ee{t}")
                            zz = hpool.tile([128, 1], F32, tag=f"zz{t}")
                            nc.scalar.activation(out=ee[:tn], in_=lg[:tn], func=mybir.ActivationFunctionType.Exp,
                                                 accum_out=zz[:tn])
                            rz = hpool.tile([128, 1], F32, tag=f"rz{t}")
                            nc.vector.reciprocal(out=rz[:tn], in_=zz[:tn])
                            nc.vector.tensor_scalar(out=ee[:tn], in0=ee[:tn], scalar1=rz[:tn], scalar2=255.0,
                                                    op0=mybir.AluOpType.mult, op1=mybir.AluOpType.mult)
                            nc.vector.tensor_scalar(out=pqb[t][:tn], in0=ee[:tn], scalar1=EXP2_23, scalar2=EXP2_23,
                                                    op0=mybir.AluOpType.add, op1=mybir.AluOpType.subtract)
                        # transpose pq -> pqT (m-major), scale by rs during evac
                        for t in range(2):          # n-tile (input partition)
                            tn = ntl[t]
                            for mt in range(2):     # m-chunk (output partition)
                                mc = ntl[mt]
                                pt = psT.tile([128, 128], BF16, tag="T")
                                nc.tensor.transpose(pt[:mc, :tn], pqb[t][:tn, noff[mt]:noff[mt] + mc], idents[:tn, :tn])
                                nc.scalar.mul(pqTs[mt][:mc, noff[t]:noff[t] + tn], pt[:mc, :tn], rs[mt][:mc, h:h + 1])
                        # PV: attnoutT_h = v_int^T-contract: out (64, NT)
                        ppv = psP.tile([64, NT], F32, tag="P")
                        for mt in range(2):
                            mc = ntl[mt]
                            nc.tensor.matmul(ppv[:], vI[mt][:mc, h * 64:(h + 1) * 64], pqTs[mt][:mc, :],
                                             start=(mt == 0), stop=(mt == 1))
                        nc.vector.tensor_copy(attnT[base:base + 64, jq, :], ppv[:])
                    # ---- proj ----
                    osb = [bpool.tile([128, DIM], F32, tag=f"osb{t}", name=f"osb{t}") for t in range(2)]
                    for t in range(2):
                        tn = ntl[t]
                        for ch in range(2):
                            pp = psT.tile([128, 384], F32, tag="T")
                            for jt in range(6):
                                nc.tensor.matmul(pp[:tn], attnT[:, jt, noff[t]:noff[t] + tn],
                                                 pws[:, jt, ch * 384:(ch + 1) * 384], start=(jt == 0), stop=(jt == 5))
                            nc.vector.scalar_tensor_tensor(out=osb[t][:tn, ch * 384:(ch + 1) * 384], in0=pp[:tn],
                                                           scalar=1.0, in1=pbs[:tn, ch * 384:(ch + 1) * 384],
                                                           op0=mybir.AluOpType.mult, op1=mybir.AluOpType.add)
                        nc.sync.dma_start(out=d["out"][tb + noff[t]:tb + noff[t] + tn, :], in_=osb[t][:tn])
    nc.compile()
    return nc


def _build_rel_index():
    H_IN = W_IN = 14
    coords = np.stack(np.meshgrid(np.arange(H_IN), np.arange(W_IN), indexing="ij"))
    flat = coords.reshape(2, -1)
    rel = flat[:, :, None] - flat[:, None, :]
    rel = rel.transpose(1, 2, 0).astype(np.int64)
    rel[:, :, 0] += H_IN - 1
    rel[:, :, 1] += W_IN - 1
    rel[:, :, 0] *= 2 * W_IN - 1
    idx = np.zeros((NT, NT), dtype=np.int64)
    idx[1:, 1:] = rel.sum(-1)
    idx[0, :] = NREL - 3
    idx[:, 0] = NREL - 2
    idx[0, 0] = NREL - 1
    return idx


def kernel(x, qkv_w, proj_w, proj_b, rel_bias_table, rel_index):
    x = np.asarray(x, dtype=np.float32)
    qkv_w = np.asarray(qkv_w, dtype=np.float32)
    proj_w = np.asarray(proj_w, dtype=np.float32)
    proj_b = np.asarray(proj_b, dtype=np.float32)
    rel_bias_table = np.asarray(rel_bias_table, dtype=np.float32)
    rel_index = np.asarray(rel_index)

    if "nc" not in _CACHE:
        _CACHE["nc"] = _build_nc()
    nc = _CACHE["nc"]

    W2 = np.ascontiguousarray(qkv_w.T)                      # (768, 2304)
    wh = W2.astype(bf)
    wl = (W2 - wh.astype(np.float32)).astype(bf)
    pw = np.ascontiguousarray(proj_w.T / 255.0).astype(bf)  # fold 1/255
    biasg = np.ascontiguousarray(
        rel_bias_table[rel_index].transpose(2, 0, 1).astype(np.float32))  # (12,197,197)
    sel = np.zeros((128, 2), np.float32)
    sel[:64, 0] = 1.0
    sel[64:, 1] = 1.0
    ident = np.eye(128, dtype=bf)

    in_maps = []
    for c in range(N_CORES):
        xc = x[c * BP:(c + 1) * BP].reshape(TOK, DIM)
        xT = np.ascontiguousarray(xc.T)                     # (768, 3152)
        xh = xT.astype(bf)
        xl = (xT - xh.astype(np.float32)).astype(bf)
        in_maps.append({
            "xh": xh, "xl": xl, "wh": wh, "wl": wl, "pw": pw,
            "pb": proj_b.astype(np.float32), "bias": biasg,
            "sel": sel, "ident": ident,
        })

    global _LAST_IN_MAPS
    _LAST_IN_MAPS = in_maps
    res = run_bass_kernel_spmd(nc, in_maps, list(range(N_CORES)))
    out = np.concatenate(
        [res.results[c]["out"].reshape(BP, NT, DIM) for c in range(N_CORES)], axis=0)
    return out.astype(np.float32)



# revision 3
# speedup vs baseline: 1.0003x; 1.0003x over previous
"""BinaryAttention Trainium2 kernel: data-parallel over batch on 8 NeuronCores.

Per-core pipeline (16 batch items):
  qkvT = W^T-split-bf16 x3 matmul (q,k transposed d-major; v token-major)
  sign/abs via ScalarE from PSUM; per-(b,h) scale c = mean|q| mean|k| /8
  S = sign(q)@sign(k)^T exact in bf16; logits = c*S + bias (DVE)
  e = exp(logits) with fused row-sum; pq = round(255*e/Z) (RNE via +-2^23)
  pqT via PE transpose, scaled by 1/s_v during PSUM evac (ScalarE)
  attnT = v_int @ pqT ; proj folds 1/255 into weights.
"""
import numpy as np
import ml_dtypes

import concourse.bacc as bacc
import concourse.mybir as mybir
from concourse.tile import TileContext
from concourse.bass_utils import run_bass_kernel_spmd
from concourse.bass import AP
import concourse.bass as bass

N_CORES = 8
B = 128
BP = B // N_CORES          # 16 batch items per core
NT = 197                   # tokens
DIM = 768
NH = 12
HD = 64
NREL = 732
TOK = BP * NT              # 3152
F32 = mybir.dt.float32
BF16 = mybir.dt.bfloat16
bf = ml_dtypes.bfloat16
EXP2_23 = 8388608.0
C0 = 1.0 / (NT * HD) / (NT * HD) / 8.0

_CACHE = {}


def _build_nc():
    nc = bacc.Bacc("TRN2", target_bir_lowering=False, debug=False, num_devices=1)
    d = {}
    d["xh"] = nc.dram_tensor("xh", [DIM, TOK], BF16, kind="ExternalInput").ap()
    d["xl"] = nc.dram_tensor("xl", [DIM, TOK], BF16, kind="ExternalInput").ap()
    d["wh"] = nc.dram_tensor("wh", [DIM, 3 * DIM], BF16, kind="ExternalInput").ap()
    d["wl"] = nc.dram_tensor("wl", [DIM, 3 * DIM], BF16, kind="ExternalInput").ap()
    d["pw"] = nc.dram_tensor("pw", [DIM, DIM], BF16, kind="ExternalInput").ap()
    d["pb"] = nc.dram_tensor("pb", [DIM], F32, kind="ExternalInput").ap()
    d["bias"] = nc.dram_tensor("bias", [NH, NT, NT], F32, kind="ExternalInput").ap()
    d["sel"] = nc.dram_tensor("sel", [128, 2], F32, kind="ExternalInput").ap()
    d["ident"] = nc.dram_tensor("ident", [128, 128], BF16, kind="ExternalInput").ap()
    d["out"] = nc.dram_tensor("out", [TOK, DIM], F32, kind="ExternalOutput").ap()
    cscr = nc.dram_tensor("cscr", [BP, 12], F32)

    with TileContext(nc) as tc:
        with (
            tc.tile_pool(name="singles", bufs=1) as singles,
            tc.tile_pool(name="xpool", bufs=3) as xpool,
            tc.tile_pool(name="bpool", bufs=2) as bpool,
            tc.tile_pool(name="hpool", bufs=5) as hpool,
            tc.tile_pool(name="psA", bufs=2, space="PSUM") as psA,
            tc.tile_pool(name="psS", bufs=2, space="PSUM") as psS,
            tc.tile_pool(name="psT", bufs=2, space="PSUM") as psT,
            tc.tile_pool(name="psP", bufs=2, space="PSUM") as psP,
        ):
            # ---- resident weights/constants ----
            whs = singles.tile([128, 6, 3 * DIM], BF16, tag="whs")
            wls = singles.tile([128, 6, 3 * DIM], BF16, tag="wls")
            nc.sync.dma_start(out=whs[:], in_=d["wh"].rearrange("(k p) n -> p k n", p=128))
            nc.sync.dma_start(out=wls[:], in_=d["wl"].rearrange("(k p) n -> p k n", p=128))
            pws = singles.tile([128, 6, DIM], BF16, tag="pws")
            nc.sync.dma_start(out=pws[:], in_=d["pw"].rearrange("(k p) n -> p k n", p=128))
            bias0 = singles.tile([128, NH, NT], F32, tag="bias0")
            bias1 = singles.tile([128, NH, NT], F32, tag="bias1")
            nc.sync.dma_start(out=bias0[:], in_=d["bias"][:, 0:128, :].rearrange("h n m -> n h m"))
            nc.sync.dma_start(out=bias1[:69], in_=d["bias"][:, 128:NT, :].rearrange("h n m -> n h m"))
            pbs = singles.tile([128, DIM], F32, tag="pbs")
            nc.gpsimd.dma_start(out=pbs[:], in_=AP(tensor=d["pb"].tensor, offset=0, ap=[[0, 128], [1, DIM]]))
            sels = singles.tile([128, 2], F32, tag="sels")
            nc.sync.dma_start(out=sels[:], in_=d["sel"])
            idents = singles.tile([128, 128], BF16, tag="idents")
            nc.sync.dma_start(out=idents[:], in_=d["ident"])

            biasn = [bias0, bias1]
            ntl = [128, 69]   # n-tile sizes
            noff = [0, 128]

            for bb in range(BP // 2):   # pairs of batch items
                c2 = 2 * NT
                xh_t = xpool.tile([128, 6, c2], BF16, tag="xh")
                xl_t = xpool.tile([128, 6, c2], BF16, tag="xl")
                nc.sync.dma_start(out=xh_t[:], in_=d["xh"].rearrange("(k p) t -> p k t", p=128)[:, :, bb * c2:(bb + 1) * c2])
                nc.sync.dma_start(out=xl_t[:], in_=d["xl"].rearrange("(k p) t -> p k t", p=128)[:, :, bb * c2:(bb + 1) * c2])

                sgn = [bpool.tile([128, NH, NT], BF16, tag=f"sgn{i}", name=f"sgn{i}") for i in range(2)]
                absc = [bpool.tile([128, 12], F32, tag=f"absc{i}", name=f"absc{i}") for i in range(2)]
                dump = [bpool.tile([128, NT], F32, tag=f"dump{i}", name=f"dump{i}") for i in range(2)]

                # ---- stage A: q,k transposed (12 j-tiles of 128 rows) ----
                for j in range(12):
                    pa = psA.tile([128, c2], F32, tag="A")
                    for k in range(6):
                        wj = slice(j * 128, (j + 1) * 128)
                        first = (k == 0)
                        nc.tensor.matmul(pa[:], whs[:, k, wj], xh_t[:, k, :], start=first, stop=False)
                        nc.tensor.matmul(pa[:], whs[:, k, wj], xl_t[:, k, :], start=False, stop=False)
                        nc.tensor.matmul(pa[:], wls[:, k, wj], xh_t[:, k, :], start=False, stop=(k == 5))
                    for i in range(2):
                        sl = slice(i * NT, (i + 1) * NT)
                        nc.scalar.activation(out=sgn[i][:, j, :], in_=pa[:, sl], func=mybir.ActivationFunctionType.Sign)
                        nc.scalar.activation(out=dump[i][:], in_=pa[:, sl], func=mybir.ActivationFunctionType.Abs,
                                             accum_out=absc[i][:, j:j + 1])

                for i in range(2):
                    b = bb * 2 + i
                    tb = b * NT   # token offset of this batch item in TOK
                    # ---- v natural + quantization ----
                    vI = [bpool.tile([128, DIM], BF16, tag=f"vI{t}", name=f"vI{t}") for t in range(2)]
                    rs = [bpool.tile([128, 12], F32, tag=f"rs{t}", name=f"rs{t}") for t in range(2)]
                    for t in range(2):
                        tn = ntl[t]
                        xoff = i * NT + noff[t]
                        pv = psA.tile([128, 384], F32, tag="A")
                        vq32 = bpool.tile([128, 384], F32, tag="vq32")
                        vmax = bpool.tile([128, 12], F32, tag="vmax")
                        ss = bpool.tile([128, 12], F32, tag="ss")
                        for ch in range(2):
                            vj = slice(1536 + ch * 384, 1536 + (ch + 1) * 384)
                            for k in range(6):
                                first = (k == 0)
                                nc.tensor.matmul(pv[:tn], xh_t[:, k, xoff:xoff + tn], whs[:, k, vj], start=first, stop=False)
                                nc.tensor.matmul(pv[:tn], xl_t[:, k, xoff:xoff + tn], whs[:, k, vj], start=False, stop=(k == 5))
                            hs = slice(ch * 6, (ch + 1) * 6)
                            # clip to [-2,2]
                            nc.vector.tensor_scalar(out=vq32[:tn], in0=pv[:tn], scalar1=2.0, scalar2=-2.0,
                                                    op0=mybir.AluOpType.min, op1=mybir.AluOpType.max)
                            # row max |.| per head
                            nc.vector.tensor_reduce(out=vmax[:tn, hs], in_=vq32[:tn].rearrange("p (h d) -> p h d", h=6),
                                                    axis=mybir.AxisListType.X, op=mybir.AluOpType.max,
                                                    apply_absolute_value=True)
                            # rs = (max+1e-8)/127 ; s = 127/(max+1e-8)
                            nc.vector.tensor_scalar(out=rs[t][:tn, hs], in0=vmax[:tn, hs], scalar1=1e-8, scalar2=1.0 / 127.0,
                                                    op0=mybir.AluOpType.add, op1=mybir.AluOpType.mult)
                            nc.vector.reciprocal(out=ss[:tn, hs], in_=rs[t][:tn, hs])
                            # v*s, round via +-2^23, cast bf16 (exact ints)
                            sbase = ss[:tn, hs]
                            sbc = AP(tensor=sbase.tensor, offset=sbase.offset,
                                     ap=[[int(s), int(c)] for s, c in sbase.ap] + [[0, HD]])
                            v3 = vq32[:tn].rearrange("p (h d) -> p h d", h=6)
                            nc.vector.tensor_tensor(out=v3, in0=v3, in1=sbc,
                                                    op=mybir.AluOpType.mult)
                            nc.vector.tensor_scalar(out=vI[t][:tn, ch * 384:(ch + 1) * 384],
                                                    in0=vq32[:tn], scalar1=EXP2_23, scalar2=EXP2_23,
                                                    op0=mybir.AluOpType.add, op1=mybir.AluOpType.subtract)
                    # ---- c stats ----
                    cst = psS.tile([2, 12], F32, tag="S")
                    nc.tensor.matmul(cst[:], sels[:], absc[i][:], start=True, stop=True)
                    css = bpool.tile([2, 12], F32, tag="css")
                    nc.vector.tensor_copy(css[:], cst[:])
                    csb = bpool.tile([2, 6], F32, tag="csb")
                    nc.vector.tensor_tensor(out=csb[:], in0=css[:2, 0:6], in1=css[:2, 6:12], op=mybir.AluOpType.mult)
                    nc.vector.tensor_scalar_mul(csb[:], csb[:], C0)
                    nc.sync.dma_start(out=cscr.ap()[b].rearrange("(r j) -> r j", r=2), in_=csb[:])
                    cbc = bpool.tile([128, 12], F32, tag="cbc")
                    nc.gpsimd.dma_start(out=cbc[:], in_=AP(tensor=cscr, offset=b * 12, ap=[[0, 128], [1, 12]]))

                    attnT = bpool.tile([128, 6, NT], BF16, tag="attnT")
                    # ---- attention per head ----
                    for h in range(12):
                        jq, base = h // 2, (h % 2) * 64
                        cidx = (h % 2) * 6 + h // 2
                        pqb = [hpool.tile([128, NT], BF16, tag=f"pqb{t}", name=f"pqb{t}") for t in range(2)]
                        pqTs = [hpool.tile([128, NT], BF16, tag=f"pqTs{t}", name=f"pqTs{t}") for t in range(2)]
                        for t in range(2):
                            tn = ntl[t]
                            ps = psS.tile([128, NT], F32, tag="S")
                            nc.tensor.matmul(ps[:tn], sgn[i][base:base + 64, jq, noff[t]:noff[t] + tn],
                                             sgn[i][base:base + 64, 6 + jq, :], start=True, stop=True)
                            lg = hpool.tile([128, NT], F32, tag=f"lg{t}")
                            nc.vector.scalar_tensor_tensor(out=lg[:tn], in0=ps[:tn], scalar=cbc[:tn, cidx:cidx + 1],
                                                           in1=biasn[t][:tn, h, :], op0=mybir.AluOpType.mult,
                                                           op1=mybir.AluOpType.add)
                            ee = hpool.tile([128, NT], F32, tag=f"ee{t}")
                            zz = hpool.tile([128, 1], F32, tag=f"zz{t}")
                            nc.scalar.activation(out=ee[:tn], in_=lg[:tn], func=mybir.ActivationFunctionType.Exp,
                                                 accum_out=zz[:tn])
                            rz = hpool.tile([128, 1], F32, tag=f"rz{t}")
                            nc.vector.reciprocal(out=rz[:tn], in_=zz[:tn])
                            nc.vector.tensor_scalar(out=ee[:tn], in0=ee[:tn], scalar1=rz[:tn], scalar2=255.0,
                                                    op0=mybir.AluOpType.mult, op1=mybir.AluOpType.mult)
                            nc.vector.tensor_scalar(out=pqb[t][:tn], in0=ee[:tn], scalar1=EXP2_23, scalar2=EXP2_23,
                                                    op0=mybir.AluOpType.add, op1=mybir.AluOpType.subtract)
                        # transpose pq -> pqT (m-major), scale by rs during evac
                        for t in range(2):          # n-tile (input partition)
                            tn = ntl[t]
                            for mt in range(2):     # m-chunk (output partition)
                                mc = ntl[mt]
                                pt = psT.tile([128, 128], BF16, tag="T")
                                nc.tensor.transpose(pt[:mc, :tn], pqb[t][:tn, noff[mt]:noff[mt] + mc], idents[:tn, :tn])
                                nc.scalar.mul(pqTs[mt][:mc, noff[t]:noff[t] + tn], pt[:mc, :tn], rs[mt][:mc, h:h + 1])
                        # PV: attnoutT_h = v_int^T-contract: out (64, NT)
                        ppv = psP.tile([64, NT], F32, tag="P")
                        for mt in range(2):
                            mc = ntl[mt]
                            nc.tensor.matmul(ppv[:], vI[mt][:mc, h * 64:(h + 1) * 64], pqTs[mt][:mc, :],
                                             start=(mt == 0), stop=(mt == 1))
                        nc.vector.tensor_copy(attnT[base:base + 64, jq, :], ppv[:])
                    # ---- proj ----
                    osb = [bpool.tile([128, DIM], F32, tag=f"osb{t}", name=f"osb{t}") for t in range(2)]
                    for t in range(2):
                        tn = ntl[t]
                        for ch in range(2):
                            pp = psT.tile([128, 384], F32, tag="T")
                            for jt in range(6):
                                nc.tensor.matmul(pp[:tn], attnT[:, jt, noff[t]:noff[t] + tn],
                                                 pws[:, jt, ch * 384:(ch + 1) * 384], start=(jt == 0), stop=(jt == 5))
                            nc.vector.scalar_tensor_tensor(out=osb[t][:tn, ch * 384:(ch + 1) * 384], in0=pp[:tn],
                                                           scalar=1.0, in1=pbs[:tn, ch * 384:(ch + 1) * 384],
                                                           op0=mybir.AluOpType.mult, op1=mybir.AluOpType.add)
                        nc.sync.dma_start(out=d["out"][tb + noff[t]:tb + noff[t] + tn, :], in_=osb[t][:tn])
    nc.compile()
    return nc


def _build_rel_index():
    H_IN = W_IN = 14
    coords = np.stack(np.meshgrid(np.arange(H_IN), np.arange(W_IN), indexing="ij"))
    flat = coords.reshape(2, -1)
    rel = flat[:, :, None] - flat[:, None, :]
    rel = rel.transpose(1, 2, 0).astype(np.int64)
    rel[:, :, 0] += H_IN - 1
    rel[:, :, 1] += W_IN - 1
    rel[:, :, 0] *= 2 * W_IN - 1
    idx = np.zeros((NT, NT), dtype=np.int64)
    idx[1:, 1:] = rel.sum(-1)
    idx[0, :] = NREL - 3
    idx[:, 0] = NREL - 2
    idx[0, 0] = NREL - 1
    return idx


def kernel(x, qkv_w, proj_w, proj_b, rel_bias_table, rel_index):
    x = np.asarray(x, dtype=np.float32)
    qkv_w = np.asarray(qkv_w, dtype=np.float32)
    proj_w = np.asarray(proj_w, dtype=np.float32)
    proj_b = np.asarray(proj_b, dtype=np.float32)
    rel_bias_table = np.asarray(rel_bias_table, dtype=np.float32)
    rel_index = np.asarray(rel_index)

    if "nc" not in _CACHE:
        _CACHE["nc"] = _build_nc()
    nc = _CACHE["nc"]

    W2 = np.ascontiguousarray(qkv_w.T)                      # (768, 2304)
    wh = W2.astype(bf)
    wl = (W2 - wh.astype(np.float32)).astype(bf)
    pw = np.ascontiguousarray(proj_w.T / 255.0).astype(bf)  # fold 1/255
    biasg = np.ascontiguousarray(
        rel_bias_table[rel_index].transpose(2, 0, 1).astype(np.float32))  # (12,197,197)
    sel = np.zeros((128, 2), np.float32)
    sel[:64, 0] = 1.0
    sel[64:, 1] = 1.0
    ident = np.eye(128, dtype=bf)

    in_maps = []
    for c in range(N_CORES):
        xc = x[c * BP:(c + 1) * BP].reshape(TOK, DIM)
        xT = np.ascontiguousarray(xc.T)                     # (768, 3152)
        xh = xT.astype(bf)
        xl = (xT - xh.astype(np.float32)).astype(bf)
        in_maps.append({
            "xh": xh, "xl": xl, "wh": wh, "wl": wl, "pw": pw,
            "pb": proj_b.astype(np.float32), "bias": biasg,
            "sel": sel, "ident": ident,
        })

    global _LAST_IN_MAPS
    _LAST_IN_MAPS = in_maps
    res = run_bass_kernel_spmd(nc, in_maps, list(range(N_CORES)))
    out = np.concatenate(
        [res.results[c]["out"].reshape(BP, NT, DIM) for c in range(N_CORES)], axis=0)
    return out.astype(np.float32)

